# revision 21
# baseline (speedup 1.0000x reference)
"""GQA attention kernel for Trainium2, 8 NeuronCores.

Problem: B=2, T=2048, E=2048, 32 query heads, 8 KV heads, head_dim=64, causal.
Sharding: 2 (batch) x 4 (tensor-parallel) grid. Each TP rank owns 2 KV heads
(=> 8 query heads, 512 q-channels). Wq/Wkv column-sharded, Wo row-sharded;
per-rank partial outputs are summed on host.

Device kernel per core (bf16 matmuls, fp32 PSUM accumulation):
  - projections: KT/VT = Wkv_loc @ x^T; V transposed to natural layout on the
    PE; QT = Wq_loc @ x^T. Projection matmuls are emitted as paced "filler"
    between attention blocks so the PE never idles (keeps HAM warm).
  - attention per (head-pair g, 512-q-window qc): both halves share one
    [128,2,512] score tile; S-matmuls use disjoint PE row groups (partition
    base 0 / 64) so they run concurrently; ONE wide exp per block on ACT;
    causal mask applied post-exp via GpSimd affine_select; PV accumulates
    [65,2,512] with a ones-row giving softmax denominators.
  - normalization: PSUM->SBUF staging (frees the ot bank early), rowsum
    broadcast via fp32 rank-1 matmul to partitions 0-63, then
    reciprocal_approx_fast + tensor_mul on DVE.
  - out_partial = att^T.T @ Wo_loc^T [T,E] fp32, summed across ranks on host.
"""

import numpy as np
import ml_dtypes

import concourse.bass as bass
import concourse.mybir as mybir
import concourse.tile as tile
from concourse import bacc
from concourse.bass_utils import run_bass_kernel_spmd

E = 2048
T = 2048
HQ = 32
HKV = 8
HD = 64
G = 4            # query heads per kv head (also: head-pairs per rank)
P = 128
QL = 512         # local q channels per rank (8 heads)
KVL = 128        # local k (or v) channels per rank (2 heads)
NB = 2           # batches
NR = 4           # tensor-parallel ranks
SCALE = 1.0 / 8.0

BF16 = mybir.dt.bfloat16
F32 = mybir.dt.float32

_CACHE = {}


def _build_program():
    from contextlib import ExitStack

    EC = E // P      # 16 contraction chunks
    TC = T // P      # 16 t chunks of 128
    T4 = T // 512    # 4 t chunks of 512

    # all inputs pre-arranged on host to partition-major so DMAs are contiguous
    nc = bacc.Bacc(None, target_bir_lowering=False, debug=False)
    xT = nc.declare_dram_parameter("xT", [P, EC * T], BF16, isOutput=False)
    wqT = nc.declare_dram_parameter("wqT", [P, EC * QL], BF16, isOutput=False)
    wkvT = nc.declare_dram_parameter("wkvT", [P, EC * 2 * KVL], BF16, isOutput=False)
    woT = nc.declare_dram_parameter("woT", [P, (QL // P) * E], BF16, isOutput=False)
    im = nc.declare_dram_parameter("im", [P, P], BF16, isOutput=False)
    out = nc.declare_dram_parameter("out", [T, E], F32, isOutput=True)

    with tile.TileContext(nc) as tc, ExitStack() as ctx:
        const = ctx.enter_context(tc.tile_pool(name="const", bufs=1))
        ptp = ctx.enter_context(tc.tile_pool(name="ptp", bufs=3))
        stg = ctx.enter_context(tc.tile_pool(name="stg", bufs=2))
        outp = ctx.enter_context(tc.tile_pool(name="outp", bufs=2))
        norm = ctx.enter_context(tc.tile_pool(name="norm", bufs=2))
        vtp = ctx.enter_context(tc.tile_pool(name="vtp", bufs=2))
        psp = ctx.enter_context(tc.tile_pool(name="psp", bufs=2, space="PSUM"))
        stp = ctx.enter_context(tc.tile_pool(name="stp", bufs=2, space="PSUM"))
        otp = ctx.enter_context(tc.tile_pool(name="otp", bufs=1, space="PSUM"))

        # ---- persistent SBUF tensors ----
        xts = const.tile([P, EC, T], BF16, tag="xts")          # x^T
        wq_s = const.tile([P, EC, QL], BF16, tag="wq")         # Wq^T (packed col order)
        wkv_s = const.tile([P, EC, 2 * KVL], BF16, tag="wkv")  # [K | V] cols
        wo_s = const.tile([P, QL // P, E], BF16, tag="wo")     # Wo^T (packed row order)
        im_s = const.tile([P, P], BF16, tag="im")              # identity (PE transpose)
        qt_s = const.tile([P, G, T], BF16, tag="qt")           # Q^T
        kt_s = const.tile([P, T], BF16, tag="kt")              # K^T
        vag_s = const.tile([P, TC, 2, 66], BF16, tag="vag")    # V_aug per (tchunk, kvhead)
        at_s = const.tile([P, G, T], BF16, tag="at")           # att out^T
        ones_s = const.tile([P, HD], BF16, tag="ones")

        # contiguous loads; x split by e-quarter so projections start after the
        # first quarter lands (groups accumulate in e order)
        nc.sync.dma_start(out=wkv_s, in_=wkvT.rearrange("p (o c) -> p o c", c=2 * KVL))
        xr = xT.rearrange("p (o t) -> p o t", t=T)
        for c in range(4):
            nc.sync.dma_start(out=xts[:, 4 * c:4 * (c + 1), :],
                              in_=xr[:, 4 * c:4 * (c + 1), :])
        wqr = wqT.rearrange("p (o q) -> p o q", q=QL)
        for c in range(2):
            nc.sync.dma_start(out=wq_s[:, 8 * c:8 * (c + 1), :],
                              in_=wqr[:, 8 * c:8 * (c + 1), :])
        nc.sync.dma_start(out=wo_s, in_=woT.rearrange("p (o e) -> p o e", e=E))
        nc.sync.dma_start(out=im_s, in_=im[:])
        nc.vector.memset(ones_s, 1.0)
        nc.vector.memset(vag_s[:, :, :, 64:66], 1.0)  # ones col (64) + pad (65)

        # ---- projection thunk builders (each thunk emits ~one instruction) ----
        def k_thunks(t4):
            ps = psp.tile([P, 512], F32, tag="ps")
            sl = slice(t4 * 512, (t4 + 1) * 512)
            ths = [
                (lambda e=e, ps=ps, sl=sl: nc.tensor.matmul(
                    ps, lhsT=wkv_s[:, e, 0:KVL], rhs=xts[:, e, sl],
                    start=(e == 0), stop=(e == EC - 1)))
                for e in range(EC)
            ]
            ths.append(lambda ps=ps, sl=sl: nc.scalar.copy(out=kt_s[:, sl], in_=ps))
            return ths

        def v_thunks(t4):
            # V^T projection then PE transpose to natural layout
            ps = psp.tile([P, 512], F32, tag="ps")
            vt = vtp.tile([P, 512], BF16, tag="vt")
            sl = slice(t4 * 512, (t4 + 1) * 512)
            ths = [
                (lambda e=e, ps=ps, sl=sl: nc.tensor.matmul(
                    ps, lhsT=wkv_s[:, e, KVL:2 * KVL], rhs=xts[:, e, sl],
                    start=(e == 0), stop=(e == EC - 1)))
                for e in range(EC)
            ]
            ths.append(lambda ps=ps, vt=vt: nc.scalar.copy(out=vt, in_=ps))
            for c in range(4):
                t = 4 * t4 + c

                def tr(c=c, t=t, vt=vt):
                    tp = psp.tile([P, P], BF16, tag="ps")
                    nc.tensor.transpose(tp, vt[:, c * P:(c + 1) * P], im_s)
                    nc.vector.tensor_copy(
                        out=vag_s[:, t, :, 0:HD],
                        in_=tp.rearrange("p (h d) -> p h d", h=2))
                ths.append(tr)
            return ths

        def q_thunks(g, t4):
            ps = psp.tile([P, 512], F32, tag="ps")
            sl = slice(t4 * 512, (t4 + 1) * 512)
            ths = [
                (lambda e=e, ps=ps, sl=sl: nc.tensor.matmul(
                    ps, lhsT=wq_s[:, e, g * P:(g + 1) * P], rhs=xts[:, e, sl],
                    start=(e == 0), stop=(e == EC - 1)))
                for e in range(EC)
            ]
            ths.append(lambda ps=ps, g=g, sl=sl: nc.vector.tensor_copy(
                out=qt_s[:, g, sl], in_=ps))
            return ths

        def o_thunks(t):
            ob = outp.tile([P, E], F32, tag="ob")
            ths = []
            for eo in range(E // 512):
                ps = psp.tile([P, 512], F32, tag="ps")
                for cc in range(QL // P):
                    ths.append(lambda ps=ps, eo=eo, cc=cc, t=t: nc.tensor.matmul(
                        ps, lhsT=at_s[:, cc, t * P:(t + 1) * P],
                        rhs=wo_s[:, cc, eo * 512:(eo + 1) * 512],
                        start=(cc == 0), stop=(cc == QL // P - 1)))
                if eo < 2:
                    ths.append(lambda ps=ps, eo=eo, ob=ob: nc.vector.tensor_copy(
                        out=ob[:, eo * 512:(eo + 1) * 512], in_=ps))
                else:
                    ths.append(lambda ps=ps, eo=eo, ob=ob: nc.scalar.copy(
                        out=ob[:, eo * 512:(eo + 1) * 512], in_=ps))
            ths.append(lambda ob=ob, t=t: nc.sync.dma_start(
                out=out.rearrange("(o p) e -> p o e", p=P)[:, t, :], in_=ob))
            return ths

        # ---- filler queue: paced between attention blocks ----
        filler = []
        fcur = [0]

        def emit_fill(n):
            stop = min(fcur[0] + n, len(filler))
            while fcur[0] < stop:
                filler[fcur[0]]()
                fcur[0] += 1

        def flush_fill():
            emit_fill(len(filler))

        # normalizations deferred so they hide under the next chain's matmuls
        pending_norm = []

        def flush_norms():
            while pending_norm:
                pending_norm.pop(0)()

        # ---- one attention chain: head pair g, q-window qc ----
        def chain(g, qc, per_block):
            q_sl = slice(qc * 512, (qc + 1) * 512)
            kmax = 4 * qc + 3
            ot = otp.tile([65, 2, 512], F32, tag="ot")
            live = {}

            def emit_S(i):
                ji = i - 4 * qc
                ci = max(ji, 0) * P
                st = stp.tile([P, 2, 512], F32, tag="st")
                pt = ptp.tile([P, 2, 512], BF16, tag="pt")
                live[i] = (st, pt, ci, ji)
                for h in range(2):
                    pb = h * HD
                    nc.tensor.matmul(
                        st[:, h, ci:512],
                        lhsT=kt_s[pb:pb + HD, i * P:(i + 1) * P],
                        rhs=qt_s[pb:pb + HD, g, qc * 512 + ci:(qc + 1) * 512],
                        start=True,
                        stop=True,
                    )

            emit_S(0)
            flush_norms()
            for i in range(kmax + 1):
                if i < kmax:
                    emit_fill(per_block)
                    emit_S(i + 1)
                st, pt, c0, j = live.pop(i)
                nc.scalar.activation(
                    out=pt[:, :, c0:512],
                    in_=st[:, :, c0:512],
                    func=mybir.ActivationFunctionType.Exp,
                    scale=SCALE,
                )
                if j >= 0:
                    # causal mask in-place: keep where q_idx >= k_partition
                    nc.gpsimd.affine_select(
                        out=pt[:, :, c0:c0 + P],
                        in_=pt[:, :, c0:c0 + P],
                        pattern=[[0, 2], [1, P]],
                        compare_op=mybir.AluOpType.is_ge,
                        fill=0.0,
                        base=0,
                        channel_multiplier=-1,
                    )
                for h in range(2):
                    nc.tensor.matmul(
                        ot[:, h, c0:512],
                        lhsT=vag_s[:, i, h, 0:65],
                        rhs=pt[:, h, c0:512],
                        start=(i == 0),
                        stop=(i == kmax),
                    )

            # evacuate PSUM early (frees ot bank for the next chain)
            sg = stg.tile([65, 2, 512], F32, tag="sg")
            nc.vector.tensor_copy(out=sg, in_=ot)
            # rowsum row to bf16 on ACT (cheap; enables bf16 broadcast matmul)
            den = norm.tile([65, 2, 512], BF16, tag="den")
            nc.scalar.copy(out=den[64:65, :, :], in_=sg[64:65, :, :])

            # normalize: broadcast rowsums to partitions 0-63 (bf16 rank-1
            # matmul), then reciprocal_approx_fast at base partition 0.
            # Deferred past the next chain's start to hide the staging latency.
            def do_norm(g=g, q_sl=q_sl, sg=sg, den=den):
                for h in range(2):
                    bcd = psp.tile([64, 512], F32, tag="ps")
                    nc.tensor.matmul(
                        bcd,
                        lhsT=ones_s[64:65, :],
                        rhs=den[64:65, h, :],
                        start=True,
                        stop=True,
                    )
                    rcs = norm.tile([64, 512], F32, tag="rcs")
                    nc.vector.reciprocal_approx_fast(out=rcs, in_=bcd)
                    if h == 0:
                        nc.vector.tensor_mul(
                            out=at_s[0:HD, g, q_sl], in0=sg[0:HD, 0, :], in1=rcs
                        )
                    else:
                        # DVE lanes can't cross partitions; write at base 0 then
                        # DMA-shift SBUF->SBUF into partitions 64..127
                        tmp = norm.tile([HD, 512], BF16, tag="tmp")
                        nc.vector.tensor_mul(out=tmp, in0=sg[0:HD, 1, :], in1=rcs)
                        nc.sync.dma_start(out=at_s[HD:P, g, q_sl], in_=tmp)
            pending_norm.append(do_norm)

        # ---- main schedule ----
        for qc in range(T4):
            # prologue at qc=0: K(0) + V(0) direct; V(1..3) go to filler
            if qc == 0:
                for th in k_thunks(0):
                    th()
                for th in v_thunks(0):
                    th()
                for t4 in range(1, T4):
                    filler.extend(v_thunks(t4))
            if qc < T4 - 1:
                filler.extend(k_thunks(qc + 1))
            if qc > 0:
                for t in range(4 * (qc - 1), 4 * qc):
                    filler.extend(o_thunks(t))

            for g in range(G):
                for th in q_thunks(g, qc):
                    th()
                blocks_left = (G - g) * (4 * qc + 3)
                remaining = len(filler) - fcur[0]
                per_block = max(1, -(-remaining // max(blocks_left, 1)))
                chain(g, qc, per_block)
            flush_fill()

        flush_norms()
        # trailing O projection for the last q-window
        for t in range(4 * (T4 - 1), 4 * T4):
            for th in o_thunks(t):
                th()

    nc.finalize()
    return nc


def _get_program():
    if "nc" not in _CACHE:
        _CACHE["nc"] = _build_program()
    return _CACHE["nc"]


def _prep_inputs(x, Wq, Wkv, Wo):
    bf = ml_dtypes.bfloat16
    x = np.asarray(x, dtype=np.float32)
    Wq = np.asarray(Wq, dtype=np.float32)
    Wkv = np.asarray(Wkv, dtype=np.float32)
    Wo = np.asarray(Wo, dtype=np.float32)

    # packed local channel order: chunk g holds [head g | head g+4]
    perm = []
    for g in range(G):
        perm.extend(range(g * HD, (g + 1) * HD))
        perm.extend(range((g + 4) * HD, (g + 5) * HD))
    perm = np.asarray(perm)

    im = np.eye(P, dtype=np.float32).astype(bf)

    def pmajor(a):
        # [E_like, F] row-grouped "(o p) f" -> partition-major [P, o*F]
        rows, f = a.shape
        o = rows // P
        return np.ascontiguousarray(
            a.reshape(o, P, f).transpose(1, 0, 2).reshape(P, o * f)).astype(bf)

    xTb = [pmajor(x[b].T) for b in range(NB)]
    wq_r, wkv_r, wo_r = [], [], []
    for r in range(NR):
        wq_loc = Wq[r * QL:(r + 1) * QL][perm]            # [512, E] packed
        wq_r.append(pmajor(np.ascontiguousarray(wq_loc.T)))
        k_rows = Wkv[r * KVL:(r + 1) * KVL]               # [128, E]
        v_rows = Wkv[HKV * HD + r * KVL:HKV * HD + (r + 1) * KVL]
        wkv_r.append(pmajor(np.concatenate([k_rows, v_rows], 0).T))
        wo_loc = Wo[:, r * QL:(r + 1) * QL][:, perm]      # [E, 512] packed cols
        wo_r.append(pmajor(np.ascontiguousarray(wo_loc.T)))

    in_maps = []
    for b in range(NB):
        for r in range(NR):
            in_maps.append({
                "xT": xTb[b],
                "wqT": wq_r[r],
                "wkvT": wkv_r[r],
                "woT": wo_r[r],
                "im": im,
            })
    return in_maps


def _run(x, Wq, Wkv, Wo, trace=False):
    nc = _get_program()
    in_maps = _prep_inputs(x, Wq, Wkv, Wo)
    res = run_bass_kernel_spmd(nc, in_maps, core_ids=list(range(8)), trace=trace)
    outs = [np.asarray(r["out"], dtype=np.float32) for r in res.results]
    full = np.stack([
        outs[0] + outs[1] + outs[2] + outs[3],
        outs[4] + outs[5] + outs[6] + outs[7],
    ]).astype(np.float32)
    return full, res


def kernel(x, Wq, Wkv, Wo):
    full, _ = _run(x, Wq, Wkv, Wo, trace=False)
    return full


# revision 26
# speedup vs baseline: 1.1629x; 1.1629x over previous
"""GQA attention kernel for Trainium2, 8 NeuronCores.

Problem: B=2, T=2048, E=2048, 32 query heads, 8 KV heads, head_dim=64, causal.
Sharding: 2 (batch) x 4 (tensor-parallel) grid. Each TP rank owns 2 KV heads
(=> 8 query heads, 512 q-channels). Wq/Wkv column-sharded, Wo row-sharded;
per-rank partial outputs are summed on host.

Device kernel per core (bf16 matmuls, fp32 PSUM accumulation):
  - projections: KT/VT = Wkv_loc @ x^T; V transposed to natural layout on the
    PE; QT = Wq_loc @ x^T. Projection matmuls are emitted as paced "filler"
    between attention blocks so the PE never idles (keeps HAM warm).
  - attention per (head-pair g, 512-q-window qc): both halves share one
    [128,2,512] score tile; S-matmuls use disjoint PE row groups (partition
    base 0 / 64) so they run concurrently; ONE wide exp per block on ACT;
    causal mask applied post-exp via GpSimd affine_select; PV accumulates
    [65,2,512] with a ones-row giving softmax denominators.
  - normalization: PSUM->SBUF staging (frees the ot bank early), rowsum
    broadcast via fp32 rank-1 matmul to partitions 0-63, then
    reciprocal_approx_fast + tensor_mul on DVE.
  - out_partial = att^T.T @ Wo_loc^T [T,E] fp32, summed across ranks on host.
"""

import numpy as np
import ml_dtypes

import concourse.bass as bass
import concourse.mybir as mybir
import concourse.tile as tile
from concourse import bacc
from concourse.bass_utils import run_bass_kernel_spmd

E = 2048
T = 2048
HQ = 32
HKV = 8
HD = 64
G = 4            # query heads per kv head (also: head-pairs per rank)
P = 128
QL = 512         # local q channels per rank (8 heads)
KVL = 128        # local k (or v) channels per rank (2 heads)
NB = 2           # batches
NR = 4           # tensor-parallel ranks
SCALE = 1.0 / 8.0

BF16 = mybir.dt.bfloat16
F32 = mybir.dt.float32

_CACHE = {}


def _build_program():
    from contextlib import ExitStack

    EC = E // P      # 16 contraction chunks
    TC = T // P      # 16 t chunks of 128
    T4 = T // 512    # 4 t chunks of 512

    # all inputs pre-arranged on host to partition-major so DMAs are contiguous
    nc = bacc.Bacc(None, target_bir_lowering=False, debug=False)
    xT = nc.declare_dram_parameter("xT", [P, EC * T], BF16, isOutput=False)
    wqT = nc.declare_dram_parameter("wqT", [P, EC * QL], BF16, isOutput=False)
    wkvT = nc.declare_dram_parameter("wkvT", [P, EC * 2 * KVL], BF16, isOutput=False)
    woT = nc.declare_dram_parameter("woT", [P, (QL // P) * E], BF16, isOutput=False)
    im = nc.declare_dram_parameter("im", [P, P], BF16, isOutput=False)
    out = nc.declare_dram_parameter("out", [T, E], F32, isOutput=True)

    with tile.TileContext(nc) as tc, ExitStack() as ctx:
        const = ctx.enter_context(tc.tile_pool(name="const", bufs=1))
        ptp = ctx.enter_context(tc.tile_pool(name="ptp", bufs=3))
        stg = ctx.enter_context(tc.tile_pool(name="stg", bufs=2))
        outp = ctx.enter_context(tc.tile_pool(name="outp", bufs=2))
        norm = ctx.enter_context(tc.tile_pool(name="norm", bufs=2))
        vtp = ctx.enter_context(tc.tile_pool(name="vtp", bufs=2))
        psp = ctx.enter_context(tc.tile_pool(name="psp", bufs=2, space="PSUM"))
        stp = ctx.enter_context(tc.tile_pool(name="stp", bufs=2, space="PSUM"))
        otp = ctx.enter_context(tc.tile_pool(name="otp", bufs=1, space="PSUM"))

        # ---- persistent SBUF tensors ----
        xts = const.tile([P, T4, EC, 512], BF16, tag="xts")    # x^T, t4-major
        wq_s = const.tile([P, G, EC, P], BF16, tag="wq")       # Wq^T, g-major
        wkv_s = const.tile([P, EC, 2 * KVL], BF16, tag="wkv")  # [K | V] cols
        wo_s = const.tile([P, QL // P, E], BF16, tag="wo")     # Wo^T (packed row order)
        im_s = const.tile([P, P], BF16, tag="im")              # identity (PE transpose)
        qt_s = const.tile([P, G, T], BF16, tag="qt")           # Q^T
        kt_s = const.tile([P, T], BF16, tag="kt")              # K^T
        vag_s = const.tile([P, TC, 2, 66], BF16, tag="vag")    # V_aug per (tchunk, kvhead)
        at_s = const.tile([P, G, T], BF16, tag="at")           # att out^T
        ones_s = const.tile([P, HD], BF16, tag="ones")

        # fully-contiguous loads (host pre-arranged); x split by t4 block and
        # wq by head-group so the first window's work starts ASAP
        nc.sync.dma_start(out=wkv_s, in_=wkvT.rearrange("p (o c) -> p o c", c=2 * KVL))
        xr = xT.rearrange("p (f o t) -> p f o t", f=T4, o=EC)
        for c in range(4):
            nc.sync.dma_start(out=xts[:, c, :, :], in_=xr[:, c, :, :])
        wqr = wqT.rearrange("p (g o q) -> p g o q", g=G, o=EC)
        for g in range(G):
            nc.sync.dma_start(out=wq_s[:, g, :, :], in_=wqr[:, g, :, :])
        nc.sync.dma_start(out=wo_s, in_=woT.rearrange("p (o e) -> p o e", e=E))
        nc.sync.dma_start(out=im_s, in_=im[:])
        nc.vector.memset(ones_s, 1.0)
        nc.vector.memset(vag_s[:, :, :, 64:66], 1.0)  # ones col (64) + pad (65)

        # ---- projection thunk builders (each thunk emits ~one instruction) ----
        def k_thunks(t4):
            ps = psp.tile([P, 512], F32, tag="ps")
            sl = slice(t4 * 512, (t4 + 1) * 512)
            ths = [
                (lambda e=e, ps=ps, t4=t4: nc.tensor.matmul(
                    ps, lhsT=wkv_s[:, e, 0:KVL], rhs=xts[:, t4, e, :],
                    start=(e == 0), stop=(e == EC - 1)))
                for e in range(EC)
            ]
            ths.append(lambda ps=ps, sl=sl: nc.scalar.copy(out=kt_s[:, sl], in_=ps))
            return ths

        def v_thunks(t4):
            # V^T projection then PE transpose to natural layout
            ps = psp.tile([P, 512], F32, tag="ps")
            vt = vtp.tile([P, 512], BF16, tag="vt")
            ths = [
                (lambda e=e, ps=ps, t4=t4: nc.tensor.matmul(
                    ps, lhsT=wkv_s[:, e, KVL:2 * KVL], rhs=xts[:, t4, e, :],
                    start=(e == 0), stop=(e == EC - 1)))
                for e in range(EC)
            ]
            ths.append(lambda ps=ps, vt=vt: nc.scalar.copy(out=vt, in_=ps))
            for c in range(4):
                t = 4 * t4 + c

                def tr(c=c, t=t, vt=vt):
                    tp = psp.tile([P, P], BF16, tag="ps")
                    nc.tensor.transpose(tp, vt[:, c * P:(c + 1) * P], im_s)
                    nc.vector.tensor_copy(
                        out=vag_s[:, t, :, 0:HD],
                        in_=tp.rearrange("p (h d) -> p h d", h=2))
                ths.append(tr)
            return ths

        def q_thunks(g, t4):
            ps = psp.tile([P, 512], F32, tag="ps")
            sl = slice(t4 * 512, (t4 + 1) * 512)
            ths = [
                (lambda e=e, ps=ps, g=g, t4=t4: nc.tensor.matmul(
                    ps, lhsT=wq_s[:, g, e, :], rhs=xts[:, t4, e, :],
                    start=(e == 0), stop=(e == EC - 1)))
                for e in range(EC)
            ]
            ths.append(lambda ps=ps, g=g, sl=sl: nc.vector.tensor_copy(
                out=qt_s[:, g, sl], in_=ps))
            return ths

        def o_thunks(t):
            ob = outp.tile([P, E], F32, tag="ob")
            ths = []
            for eo in range(E // 512):
                ps = psp.tile([P, 512], F32, tag="ps")
                for cc in range(QL // P):
                    ths.append(lambda ps=ps, eo=eo, cc=cc, t=t: nc.tensor.matmul(
                        ps, lhsT=at_s[:, cc, t * P:(t + 1) * P],
                        rhs=wo_s[:, cc, eo * 512:(eo + 1) * 512],
                        start=(cc == 0), stop=(cc == QL // P - 1)))
                if eo < 2:
                    ths.append(lambda ps=ps, eo=eo, ob=ob: nc.vector.tensor_copy(
                        out=ob[:, eo * 512:(eo + 1) * 512], in_=ps))
                else:
                    ths.append(lambda ps=ps, eo=eo, ob=ob: nc.scalar.copy(
                        out=ob[:, eo * 512:(eo + 1) * 512], in_=ps))
            ths.append(lambda ob=ob, t=t: nc.sync.dma_start(
                out=out.rearrange("(o p) e -> p o e", p=P)[:, t, :], in_=ob))
            return ths

        # ---- filler queue: paced between attention blocks ----
        filler = []
        fcur = [0]

        def emit_fill(n):
            stop = min(fcur[0] + n, len(filler))
            while fcur[0] < stop:
                filler[fcur[0]]()
                fcur[0] += 1

        def flush_fill():
            emit_fill(len(filler))

        # normalizations deferred so they hide under the next chain's matmuls
        pending_norm = []

        def flush_norms():
            while pending_norm:
                pending_norm.pop(0)()

        # ---- one attention chain: head pair g, q-window qc ----
        def chain(g, qc, per_block):
            q_sl = slice(qc * 512, (qc + 1) * 512)
            kmax = 4 * qc + 3
            ot = otp.tile([65, 2, 512], F32, tag="ot")
            live = {}

            def emit_S(i):
                ji = i - 4 * qc
                ci = max(ji, 0) * P
                st = stp.tile([P, 2, 512], F32, tag="st")
                pt = ptp.tile([P, 2, 512], BF16, tag="pt")
                live[i] = (st, pt, ci, ji)
                for h in range(2):
                    pb = h * HD
                    nc.tensor.matmul(
                        st[:, h, ci:512],
                        lhsT=kt_s[pb:pb + HD, i * P:(i + 1) * P],
                        rhs=qt_s[pb:pb + HD, g, qc * 512 + ci:(qc + 1) * 512],
                        start=True,
                        stop=True,
                    )

            emit_S(0)
            flush_norms()
            for i in range(kmax + 1):
                if i < kmax:
                    emit_fill(per_block)
                    emit_S(i + 1)
                st, pt, c0, j = live.pop(i)
                nc.scalar.activation(
                    out=pt[:, :, c0:512],
                    in_=st[:, :, c0:512],
                    func=mybir.ActivationFunctionType.Exp,
                    scale=SCALE,
                )
                if j >= 0:
                    # causal mask in-place: keep where q_idx >= k_partition
                    nc.gpsimd.affine_select(
                        out=pt[:, :, c0:c0 + P],
                        in_=pt[:, :, c0:c0 + P],
                        pattern=[[0, 2], [1, P]],
                        compare_op=mybir.AluOpType.is_ge,
                        fill=0.0,
                        base=0,
                        channel_multiplier=-1,
                    )
                for h in range(2):
                    nc.tensor.matmul(
                        ot[:, h, c0:512],
                        lhsT=vag_s[:, i, h, 0:65],
                        rhs=pt[:, h, c0:512],
                        start=(i == 0),
                        stop=(i == kmax),
                    )

            # evacuate PSUM early (frees ot bank for the next chain)
            sg = stg.tile([65, 2, 512], F32, tag="sg")
            nc.vector.tensor_copy(out=sg, in_=ot)
            # rowsum row to bf16 on ACT (cheap; enables bf16 broadcast matmul)
            den = norm.tile([65, 2, 512], BF16, tag="den")
            nc.scalar.copy(out=den[64:65, :, :], in_=sg[64:65, :, :])

            # normalize: broadcast rowsums to partitions 0-63 (bf16 rank-1
            # matmul), then reciprocal_approx_fast at base partition 0.
            # Deferred past the next chain's start to hide the staging latency.
            def do_norm(g=g, q_sl=q_sl, sg=sg, den=den):
                for h in range(2):
                    bcd = psp.tile([64, 512], F32, tag="ps")
                    nc.tensor.matmul(
                        bcd,
                        lhsT=ones_s[64:65, :],
                        rhs=den[64:65, h, :],
                        start=True,
                        stop=True,
                    )
                    rcs = norm.tile([64, 512], F32, tag="rcs")
                    nc.vector.reciprocal_approx_fast(out=rcs, in_=bcd)
                    if h == 0:
                        nc.vector.tensor_mul(
                            out=at_s[0:HD, g, q_sl], in0=sg[0:HD, 0, :], in1=rcs
                        )
                    else:
                        # DVE lanes can't cross partitions; write at base 0 then
                        # DMA-shift SBUF->SBUF into partitions 64..127
                        tmp = norm.tile([HD, 512], BF16, tag="tmp")
                        nc.vector.tensor_mul(out=tmp, in0=sg[0:HD, 1, :], in1=rcs)
                        nc.sync.dma_start(out=at_s[HD:P, g, q_sl], in_=tmp)
            pending_norm.append(do_norm)

        # ---- main schedule ----
        for qc in range(T4):
            # prologue at qc=0: K(0) + V(0) direct; V(1..3) go to filler
            if qc == 0:
                for th in k_thunks(0):
                    th()
                for th in v_thunks(0):
                    th()
                for t4 in range(1, T4):
                    filler.extend(v_thunks(t4))
            if qc < T4 - 1:
                filler.extend(k_thunks(qc + 1))
            if qc > 0:
                for t in range(4 * (qc - 1), 4 * qc):
                    filler.extend(o_thunks(t))

            for g in range(G):
                for th in q_thunks(g, qc):
                    th()
                blocks_left = (G - g) * (4 * qc + 3)
                remaining = len(filler) - fcur[0]
                per_block = max(1, -(-remaining // max(blocks_left, 1)))
                chain(g, qc, per_block)
            flush_fill()

        flush_norms()
        # trailing O projection for the last q-window
        for t in range(4 * (T4 - 1), 4 * T4):
            for th in o_thunks(t):
                th()

    nc.finalize()
    return nc


def _get_program():
    if "nc" not in _CACHE:
        _CACHE["nc"] = _build_program()
    return _CACHE["nc"]


def _prep_inputs(x, Wq, Wkv, Wo):
    bf = ml_dtypes.bfloat16
    x = np.asarray(x, dtype=np.float32)
    Wq = np.asarray(Wq, dtype=np.float32)
    Wkv = np.asarray(Wkv, dtype=np.float32)
    Wo = np.asarray(Wo, dtype=np.float32)

    # packed local channel order: chunk g holds [head g | head g+4]
    perm = []
    for g in range(G):
        perm.extend(range(g * HD, (g + 1) * HD))
        perm.extend(range((g + 4) * HD, (g + 5) * HD))
    perm = np.asarray(perm)

    im = np.eye(P, dtype=np.float32).astype(bf)

    def pmajor(a):
        # [E_like, F] row-grouped "(o p) f" -> partition-major [P, o*F]
        rows, f = a.shape
        o = rows // P
        return np.ascontiguousarray(
            a.reshape(o, P, f).transpose(1, 0, 2).reshape(P, o * f)).astype(bf)

    def pmajor_x(a):
        # x^T [E, T] -> [P, T4, EC, 512] t4-major contiguous blocks
        return np.ascontiguousarray(
            a.reshape(E // P, P, T // 512, 512).transpose(1, 2, 0, 3)
             .reshape(P, (E // P) * T)).astype(bf)

    def pmajor_wq(a):
        # Wq^T [E, QL] -> [P, G, EC, 128] g-major contiguous blocks
        return np.ascontiguousarray(
            a.reshape(E // P, P, G, P).transpose(1, 2, 0, 3)
             .reshape(P, (E // P) * QL)).astype(bf)

    xTb = [pmajor_x(x[b].T) for b in range(NB)]
    wq_r, wkv_r, wo_r = [], [], []
    for r in range(NR):
        wq_loc = Wq[r * QL:(r + 1) * QL][perm]            # [512, E] packed
        wq_r.append(pmajor_wq(np.ascontiguousarray(wq_loc.T)))
        k_rows = Wkv[r * KVL:(r + 1) * KVL]               # [128, E]
        v_rows = Wkv[HKV * HD + r * KVL:HKV * HD + (r + 1) * KVL]
        wkv_r.append(pmajor(np.concatenate([k_rows, v_rows], 0).T))
        wo_loc = Wo[:, r * QL:(r + 1) * QL][:, perm]      # [E, 512] packed cols
        wo_r.append(pmajor(np.ascontiguousarray(wo_loc.T)))

    in_maps = []
    for b in range(NB):
        for r in range(NR):
            in_maps.append({
                "xT": xTb[b],
                "wqT": wq_r[r],
                "wkvT": wkv_r[r],
                "woT": wo_r[r],
                "im": im,
            })
    return in_maps


def _run(x, Wq, Wkv, Wo, trace=False):
    nc = _get_program()
    in_maps = _prep_inputs(x, Wq, Wkv, Wo)
    res = run_bass_kernel_spmd(nc, in_maps, core_ids=list(range(8)), trace=trace)
    outs = [np.asarray(r["out"], dtype=np.float32) for r in res.results]
    full = np.stack([
        outs[0] + outs[1] + outs[2] + outs[3],
        outs[4] + outs[5] + outs[6] + outs[7],
    ]).astype(np.float32)
    return full, res


def kernel(x, Wq, Wkv, Wo):
    full, _ = _run(x, Wq, Wkv, Wo, trace=False)
    return full


# revision 27
# speedup vs baseline: 1.2415x; 1.0676x over previous
"""GQA attention kernel for Trainium2, 8 NeuronCores.

Problem: B=2, T=2048, E=2048, 32 query heads, 8 KV heads, head_dim=64, causal.
Sharding: 2 (batch) x 4 (tensor-parallel) grid. Each TP rank owns 2 KV heads
(=> 8 query heads, 512 q-channels). Wq/Wkv column-sharded, Wo row-sharded;
per-rank partial outputs are summed on host.

Device kernel per core (bf16 matmuls, fp32 PSUM accumulation):
  - projections: KT/VT = Wkv_loc @ x^T; V transposed to natural layout on the
    PE; QT = Wq_loc @ x^T. Projection matmuls are emitted as paced "filler"
    between attention blocks so the PE never idles (keeps HAM warm).
  - attention per (head-pair g, 512-q-window qc): both halves share one
    [128,2,512] score tile; S-matmuls use disjoint PE row groups (partition
    base 0 / 64) so they run concurrently; ONE wide exp per block on ACT;
    causal mask applied post-exp via GpSimd affine_select; PV accumulates
    [65,2,512] with a ones-row giving softmax denominators.
  - normalization: PSUM->SBUF staging (frees the ot bank early), rowsum
    broadcast via fp32 rank-1 matmul to partitions 0-63, then
    reciprocal_approx_fast + tensor_mul on DVE.
  - out_partial = att^T.T @ Wo_loc^T [T,E] fp32, summed across ranks on host.
"""

import numpy as np
import ml_dtypes

import concourse.bass as bass
import concourse.mybir as mybir
import concourse.tile as tile
from concourse import bacc
from concourse.bass_utils import run_bass_kernel_spmd

E = 2048
T = 2048
HQ = 32
HKV = 8
HD = 64
G = 4            # query heads per kv head (also: head-pairs per rank)
P = 128
QL = 512         # local q channels per rank (8 heads)
KVL = 128        # local k (or v) channels per rank (2 heads)
NB = 2           # batches
NR = 4           # tensor-parallel ranks
SCALE = 1.0 / 8.0

BF16 = mybir.dt.bfloat16
F32 = mybir.dt.float32

_CACHE = {}


def _build_program():
    from contextlib import ExitStack

    EC = E // P      # 16 contraction chunks
    TC = T // P      # 16 t chunks of 128
    T4 = T // 512    # 4 t chunks of 512

    # all inputs pre-arranged on host to partition-major so DMAs are contiguous
    nc = bacc.Bacc(None, target_bir_lowering=False, debug=False)
    xT = nc.declare_dram_parameter("xT", [P, EC * T], BF16, isOutput=False)
    wqT = nc.declare_dram_parameter("wqT", [P, EC * QL], BF16, isOutput=False)
    wkvT = nc.declare_dram_parameter("wkvT", [P, EC * 2 * KVL], BF16, isOutput=False)
    woT = nc.declare_dram_parameter("woT", [P, (QL // P) * E], BF16, isOutput=False)
    im = nc.declare_dram_parameter("im", [P, P], BF16, isOutput=False)
    out = nc.declare_dram_parameter("out", [T, E], F32, isOutput=True)

    with tile.TileContext(nc) as tc, ExitStack() as ctx:
        const = ctx.enter_context(tc.tile_pool(name="const", bufs=1))
        ptp = ctx.enter_context(tc.tile_pool(name="ptp", bufs=3))
        stg = ctx.enter_context(tc.tile_pool(name="stg", bufs=2))
        outp = ctx.enter_context(tc.tile_pool(name="outp", bufs=2))
        norm = ctx.enter_context(tc.tile_pool(name="norm", bufs=2))
        vtp = ctx.enter_context(tc.tile_pool(name="vtp", bufs=2))
        psp = ctx.enter_context(tc.tile_pool(name="psp", bufs=2, space="PSUM"))
        stp = ctx.enter_context(tc.tile_pool(name="stp", bufs=2, space="PSUM"))
        otp = ctx.enter_context(tc.tile_pool(name="otp", bufs=1, space="PSUM"))

        # ---- persistent SBUF tensors ----
        xts = const.tile([P, T4, EC, 512], BF16, tag="xts")    # x^T, t4-major
        wq_s = const.tile([P, G, EC, P], BF16, tag="wq")       # Wq^T, g-major
        wkv_s = const.tile([P, EC, 2 * KVL], BF16, tag="wkv")  # [K | V] cols
        wo_s = const.tile([P, QL // P, E], BF16, tag="wo")     # Wo^T (packed row order)
        im_s = const.tile([P, P], BF16, tag="im")              # identity (PE transpose)
        qt_s = const.tile([P, G, T], BF16, tag="qt")           # Q^T
        kt_s = const.tile([P, T], BF16, tag="kt")              # K^T
        vag_s = const.tile([P, TC, 2, 66], BF16, tag="vag")    # V_aug per (tchunk, kvhead)
        at_s = const.tile([P, G, T], BF16, tag="at")           # att out^T
        ones_s = const.tile([P, HD], BF16, tag="ones")

        # fully-contiguous loads (host pre-arranged); x split by t4 block and
        # wq by head-group, ordered so the first window's work starts ASAP
        xr = xT.rearrange("p (f o t) -> p f o t", f=T4, o=EC)
        wqr = wqT.rearrange("p (g o q) -> p g o q", g=G, o=EC)
        nc.sync.dma_start(out=wkv_s, in_=wkvT.rearrange("p (o c) -> p o c", c=2 * KVL))
        nc.sync.dma_start(out=xts[:, 0, :, :], in_=xr[:, 0, :, :])
        nc.sync.dma_start(out=wq_s[:, 0, :, :], in_=wqr[:, 0, :, :])
        nc.sync.dma_start(out=im_s, in_=im[:])
        for c in range(1, 4):
            nc.sync.dma_start(out=xts[:, c, :, :], in_=xr[:, c, :, :])
        for g in range(1, G):
            nc.sync.dma_start(out=wq_s[:, g, :, :], in_=wqr[:, g, :, :])
        nc.sync.dma_start(out=wo_s, in_=woT.rearrange("p (o e) -> p o e", e=E))
        nc.vector.memset(ones_s, 1.0)
        nc.vector.memset(vag_s[:, :, :, 64:66], 1.0)  # ones col (64) + pad (65)

        # ---- projection thunk builders (each thunk emits ~one instruction) ----
        def k_thunks(t4):
            ps = psp.tile([P, 512], F32, tag="ps")
            sl = slice(t4 * 512, (t4 + 1) * 512)
            ths = [
                (lambda e=e, ps=ps, t4=t4: nc.tensor.matmul(
                    ps, lhsT=wkv_s[:, e, 0:KVL], rhs=xts[:, t4, e, :],
                    start=(e == 0), stop=(e == EC - 1)))
                for e in range(EC)
            ]
            ths.append(lambda ps=ps, sl=sl: nc.scalar.copy(out=kt_s[:, sl], in_=ps))
            return ths

        def v_thunks(t4):
            # V^T projection then PE transpose to natural layout
            ps = psp.tile([P, 512], F32, tag="ps")
            vt = vtp.tile([P, 512], BF16, tag="vt")
            ths = [
                (lambda e=e, ps=ps, t4=t4: nc.tensor.matmul(
                    ps, lhsT=wkv_s[:, e, KVL:2 * KVL], rhs=xts[:, t4, e, :],
                    start=(e == 0), stop=(e == EC - 1)))
                for e in range(EC)
            ]
            ths.append(lambda ps=ps, vt=vt: nc.scalar.copy(out=vt, in_=ps))
            for c in range(4):
                t = 4 * t4 + c

                def tr(c=c, t=t, vt=vt):
                    tp = psp.tile([P, P], BF16, tag="ps")
                    nc.tensor.transpose(tp, vt[:, c * P:(c + 1) * P], im_s)
                    nc.vector.tensor_copy(
                        out=vag_s[:, t, :, 0:HD],
                        in_=tp.rearrange("p (h d) -> p h d", h=2))
                ths.append(tr)
            return ths

        def q_thunks(g, t4):
            ps = psp.tile([P, 512], F32, tag="ps")
            sl = slice(t4 * 512, (t4 + 1) * 512)
            ths = [
                (lambda e=e, ps=ps, g=g, t4=t4: nc.tensor.matmul(
                    ps, lhsT=wq_s[:, g, e, :], rhs=xts[:, t4, e, :],
                    start=(e == 0), stop=(e == EC - 1)))
                for e in range(EC)
            ]
            ths.append(lambda ps=ps, g=g, sl=sl: nc.vector.tensor_copy(
                out=qt_s[:, g, sl], in_=ps))
            return ths

        def o_thunks(t):
            ob = outp.tile([P, E], F32, tag="ob")
            ths = []
            for eo in range(E // 512):
                ps = psp.tile([P, 512], F32, tag="ps")
                for cc in range(QL // P):
                    ths.append(lambda ps=ps, eo=eo, cc=cc, t=t: nc.tensor.matmul(
                        ps, lhsT=at_s[:, cc, t * P:(t + 1) * P],
                        rhs=wo_s[:, cc, eo * 512:(eo + 1) * 512],
                        start=(cc == 0), stop=(cc == QL // P - 1)))
                if eo < 2:
                    ths.append(lambda ps=ps, eo=eo, ob=ob: nc.vector.tensor_copy(
                        out=ob[:, eo * 512:(eo + 1) * 512], in_=ps))
                else:
                    ths.append(lambda ps=ps, eo=eo, ob=ob: nc.scalar.copy(
                        out=ob[:, eo * 512:(eo + 1) * 512], in_=ps))
            ths.append(lambda ob=ob, t=t: nc.sync.dma_start(
                out=out.rearrange("(o p) e -> p o e", p=P)[:, t, :], in_=ob))
            return ths

        # ---- filler queue: paced between attention blocks ----
        filler = []
        fcur = [0]

        def emit_fill(n):
            stop = min(fcur[0] + n, len(filler))
            while fcur[0] < stop:
                filler[fcur[0]]()
                fcur[0] += 1

        def flush_fill():
            emit_fill(len(filler))

        # normalizations deferred so they hide under the next chain's matmuls
        pending_norm = []

        def flush_norms():
            while pending_norm:
                pending_norm.pop(0)()

        # ---- one attention chain: head pair g, q-window qc ----
        def chain(g, qc, per_block):
            q_sl = slice(qc * 512, (qc + 1) * 512)
            kmax = 4 * qc + 3
            ot = otp.tile([65, 2, 512], F32, tag="ot")
            live = {}

            def emit_S(i):
                ji = i - 4 * qc
                ci = max(ji, 0) * P
                st = stp.tile([P, 2, 512], F32, tag="st")
                pt = ptp.tile([P, 2, 512], BF16, tag="pt")
                live[i] = (st, pt, ci, ji)
                for h in range(2):
                    pb = h * HD
                    nc.tensor.matmul(
                        st[:, h, ci:512],
                        lhsT=kt_s[pb:pb + HD, i * P:(i + 1) * P],
                        rhs=qt_s[pb:pb + HD, g, qc * 512 + ci:(qc + 1) * 512],
                        start=True,
                        stop=True,
                    )

            emit_S(0)
            flush_norms()
            for i in range(kmax + 1):
                if i < kmax:
                    emit_fill(per_block)
                    emit_S(i + 1)
                st, pt, c0, j = live.pop(i)
                nc.scalar.activation(
                    out=pt[:, :, c0:512],
                    in_=st[:, :, c0:512],
                    func=mybir.ActivationFunctionType.Exp,
                    scale=SCALE,
                )
                if j >= 0:
                    # causal mask in-place: keep where q_idx >= k_partition
                    nc.gpsimd.affine_select(
                        out=pt[:, :, c0:c0 + P],
                        in_=pt[:, :, c0:c0 + P],
                        pattern=[[0, 2], [1, P]],
                        compare_op=mybir.AluOpType.is_ge,
                        fill=0.0,
                        base=0,
                        channel_multiplier=-1,
                    )
                for h in range(2):
                    nc.tensor.matmul(
                        ot[:, h, c0:512],
                        lhsT=vag_s[:, i, h, 0:65],
                        rhs=pt[:, h, c0:512],
                        start=(i == 0),
                        stop=(i == kmax),
                    )

            # evacuate PSUM early (frees ot bank for the next chain)
            sg = stg.tile([65, 2, 512], F32, tag="sg")
            nc.vector.tensor_copy(out=sg, in_=ot)
            # rowsum row to bf16 on ACT (cheap; enables bf16 broadcast matmul)
            den = norm.tile([65, 2, 512], BF16, tag="den")
            nc.scalar.copy(out=den[64:65, :, :], in_=sg[64:65, :, :])

            # normalize: broadcast rowsums to partitions 0-63 (bf16 rank-1
            # matmul), then reciprocal_approx_fast at base partition 0.
            # Deferred past the next chain's start to hide the staging latency.
            def do_norm(g=g, q_sl=q_sl, sg=sg, den=den):
                for h in range(2):
                    bcd = psp.tile([64, 512], F32, tag="ps")
                    nc.tensor.matmul(
                        bcd,
                        lhsT=ones_s[64:65, :],
                        rhs=den[64:65, h, :],
                        start=True,
                        stop=True,
                    )
                    rcs = norm.tile([64, 512], F32, tag="rcs")
                    nc.vector.reciprocal_approx_fast(out=rcs, in_=bcd)
                    if h == 0:
                        nc.vector.tensor_mul(
                            out=at_s[0:HD, g, q_sl], in0=sg[0:HD, 0, :], in1=rcs
                        )
                    else:
                        # DVE lanes can't cross partitions; write at base 0 then
                        # DMA-shift SBUF->SBUF into partitions 64..127
                        tmp = norm.tile([HD, 512], BF16, tag="tmp")
                        nc.vector.tensor_mul(out=tmp, in0=sg[0:HD, 1, :], in1=rcs)
                        nc.sync.dma_start(out=at_s[HD:P, g, q_sl], in_=tmp)
            pending_norm.append(do_norm)

        # ---- main schedule ----
        for qc in range(T4):
            # prologue at qc=0: K(0) + V(0) direct; V(1..3) go to filler
            if qc == 0:
                for th in k_thunks(0):
                    th()
                for th in v_thunks(0):
                    th()
                for t4 in range(1, T4):
                    filler.extend(v_thunks(t4))
            if qc < T4 - 1:
                filler.extend(k_thunks(qc + 1))
            if qc > 0:
                for t in range(4 * (qc - 1), 4 * qc):
                    filler.extend(o_thunks(t))

            for g in range(G):
                for th in q_thunks(g, qc):
                    th()
                blocks_left = (G - g) * (4 * qc + 3)
                remaining = len(filler) - fcur[0]
                per_block = max(1, -(-remaining // max(blocks_left, 1)))
                chain(g, qc, per_block)
            flush_fill()

        flush_norms()
        # trailing O projection for the last q-window
        for t in range(4 * (T4 - 1), 4 * T4):
            for th in o_thunks(t):
                th()

    nc.finalize()
    return nc


def _get_program():
    if "nc" not in _CACHE:
        _CACHE["nc"] = _build_program()
    return _CACHE["nc"]


def _prep_inputs(x, Wq, Wkv, Wo):
    bf = ml_dtypes.bfloat16
    x = np.asarray(x, dtype=np.float32)
    Wq = np.asarray(Wq, dtype=np.float32)
    Wkv = np.asarray(Wkv, dtype=np.float32)
    Wo = np.asarray(Wo, dtype=np.float32)

    # packed local channel order: chunk g holds [head g | head g+4]
    perm = []
    for g in range(G):
        perm.extend(range(g * HD, (g + 1) * HD))
        perm.extend(range((g + 4) * HD, (g + 5) * HD))
    perm = np.asarray(perm)

    im = np.eye(P, dtype=np.float32).astype(bf)

    def pmajor(a):
        # [E_like, F] row-grouped "(o p) f" -> partition-major [P, o*F]
        rows, f = a.shape
        o = rows // P
        return np.ascontiguousarray(
            a.reshape(o, P, f).transpose(1, 0, 2).reshape(P, o * f)).astype(bf)

    def pmajor_x(a):
        # x^T [E, T] -> [P, T4, EC, 512] t4-major contiguous blocks
        return np.ascontiguousarray(
            a.reshape(E // P, P, T // 512, 512).transpose(1, 2, 0, 3)
             .reshape(P, (E // P) * T)).astype(bf)

    def pmajor_wq(a):
        # Wq^T [E, QL] -> [P, G, EC, 128] g-major contiguous blocks
        return np.ascontiguousarray(
            a.reshape(E // P, P, G, P).transpose(1, 2, 0, 3)
             .reshape(P, (E // P) * QL)).astype(bf)

    xTb = [pmajor_x(x[b].T) for b in range(NB)]
    wq_r, wkv_r, wo_r = [], [], []
    for r in range(NR):
        wq_loc = Wq[r * QL:(r + 1) * QL][perm]            # [512, E] packed
        wq_r.append(pmajor_wq(np.ascontiguousarray(wq_loc.T)))
        k_rows = Wkv[r * KVL:(r + 1) * KVL]               # [128, E]
        v_rows = Wkv[HKV * HD + r * KVL:HKV * HD + (r + 1) * KVL]
        wkv_r.append(pmajor(np.concatenate([k_rows, v_rows], 0).T))
        wo_loc = Wo[:, r * QL:(r + 1) * QL][:, perm]      # [E, 512] packed cols
        wo_r.append(pmajor(np.ascontiguousarray(wo_loc.T)))

    in_maps = []
    for b in range(NB):
        for r in range(NR):
            in_maps.append({
                "xT": xTb[b],
                "wqT": wq_r[r],
                "wkvT": wkv_r[r],
                "woT": wo_r[r],
                "im": im,
            })
    return in_maps


def _run(x, Wq, Wkv, Wo, trace=False):
    nc = _get_program()
    in_maps = _prep_inputs(x, Wq, Wkv, Wo)
    res = run_bass_kernel_spmd(nc, in_maps, core_ids=list(range(8)), trace=trace)
    outs = [np.asarray(r["out"], dtype=np.float32) for r in res.results]
    full = np.stack([
        outs[0] + outs[1] + outs[2] + outs[3],
        outs[4] + outs[5] + outs[6] + outs[7],
    ]).astype(np.float32)
    return full, res


def kernel(x, Wq, Wkv, Wo):
    full, _ = _run(x, Wq, Wkv, Wo, trace=False)
    return full
